# revision 2
# baseline (speedup 1.0000x reference)
"""Amortized-VI loss kernel for 8 TRN2 NeuronCores (data-parallel).

Reference computation (see problem): 3 encoder MLPs on y -> (mu, Ldiag, Loff),
Cholesky-reparameterized xi = mu + L z, a DenseResNet PINN evaluated at 9 x-ticks
per (sample, particle), Gaussian log-likelihood + prior + entropy, mean-reduced
to one scalar.

Device strategy (per core, n_loc=1024 samples):
  - feature-on-partition layout; 3-way block-diagonal weight packing so the
    128-wide engines see [120, N] tiles (3 chunks x 40 features).
  - PINN rows are ordered tick-major: global row = t*8192 + (n*8+p).
    Chunk c covers ticks 3c..3c+2; 12 supertiles of 2048 columns; supertile s
    has tick-group g=s//4 and xi-window w=s%4 (columns 2048w..2048w+2048 of the
    8192 xi rows), shared by all 3 chunks.
  - The x-tick input coordinate folds into a per-(chunk,group) bias on layer 1;
    the output layer folds the likelihood residual: D = y - a_t - c_t*b_out is
    precomputed on host and d = c_t*net_raw - D accumulates into sum(d^2).
  - Loss reduces to three sums (resid^2, xi^2, ln softplus) finished on host.
"""

import numpy as np

D_XI = 5
N_TICKS = 9
NOISE = 0.015
LOG2PI = float(np.log(2.0 * np.pi))
X_TICKS = np.linspace(0.15, 0.85, N_TICKS).astype(np.float64)
B1, B2 = 1.0, 0.0

NCORES = 8
NLOC = 1024            # samples per core
P = 8                  # particles
NXI = NLOC * P         # 8192 xi rows per core
CH = 3                 # chunks (3 ticks each)
ST = 12                # supertiles per core
FD = 2048              # supertile free dim
TI, TJ = np.tril_indices(D_XI, -1)   # 10 strict-lower pairs (reference order)
NWIN = 4               # xi windows


def _as_np(x):
    return np.asarray(x, dtype=np.float32)


def _f16(x):
    return np.ascontiguousarray(np.asarray(x, dtype=np.float32).astype(np.float16))


def _f32(x):
    return np.ascontiguousarray(np.asarray(x, dtype=np.float32))


def _blockdiag(mats):
    rows = sum(m.shape[0] for m in mats)
    cols = sum(m.shape[1] for m in mats)
    out = np.zeros((rows, cols), np.float32)
    r = c = 0
    for m in mats:
        out[r:r + m.shape[0], c:c + m.shape[1]] = m
        r += m.shape[0]
        c += m.shape[1]
    return out


def _pad_cols(m, cols=128):
    out = np.zeros((m.shape[0], cols), np.float32)
    out[:, :m.shape[1]] = m
    return out


def _prep_shared(mu_params, ldiag_params, loff_params, pinn_params):
    """Host-packed weights shared by all cores. Returns dict name->np array."""
    mu_p = [(_as_np(W), _as_np(b)) for (W, b) in mu_params]
    ld_p = [(_as_np(W), _as_np(b)) for (W, b) in ldiag_params]
    lo_p = [(_as_np(W), _as_np(b)) for (W, b) in loff_params]
    W_in, b_in = _as_np(pinn_params["inp"][0]), _as_np(pinn_params["inp"][1])
    blocks = [[(_as_np(W), _as_np(b)) for (W, b) in blk]
              for blk in pinn_params["blocks"]]
    W_out, b_out = _as_np(pinn_params["out"][0]), _as_np(pinn_params["out"][1])
    w_out = W_out[:, 0]          # [40]
    b_out = float(b_out[0])

    d = {}
    # mu trunk (5 layers)
    for li, (W, b) in enumerate(mu_p):
        d[f"mW{li}"] = _f16(W)
        d[f"mB{li}"] = _f32(b[:, None])
    # packed ldiag+loff trunk
    for li in range(5):
        d[f"pW{li}"] = _f16(_blockdiag([ld_p[li][0], lo_p[li][0]]))
        d[f"pB{li}"] = _f32(np.concatenate([ld_p[li][1], lo_p[li][1]])[:, None])
    # G matmuls: A (from Mprod rows) and B (from mu), both [K, 128pad]
    W1 = W_in[1:, :]             # [5, 40]
    i_of_row = np.concatenate([np.arange(D_XI), TI])   # [15]
    A = np.zeros((15, 3 * 40), np.float32)
    Bm = np.zeros((5, 3 * 40), np.float32)
    for c in range(3):
        for r in range(15):
            A[r, c * 40:(c + 1) * 40] = W1[i_of_row[r]]
        Bm[:, c * 40:(c + 1) * 40] = W1
    d["AG"] = _f16(_pad_cols(A))
    d["BG"] = _f16(_pad_cols(Bm))
    # xi selection for the prior: xi = Sxi^T Mprod + I^T mu
    Sxi = np.zeros((15, D_XI), np.float32)
    for r in range(15):
        Sxi[r, i_of_row[r]] = 1.0
    d["SXI"] = _f16(Sxi)
    d["I5"] = _f16(np.eye(D_XI, dtype=np.float32))
    # h1 biases per (group): [120,1], rows c*40+f = b_in[f] + W_in[0,f]*x_t(c,g)
    for g in range(3):
        bb = np.zeros((120, 1), np.float32)
        for c in range(3):
            t = 3 * c + g
            bb[c * 40:(c + 1) * 40, 0] = b_in + W_in[0, :] * X_TICKS[t]
        d[f"b1g{g}"] = _f32(bb)
    # hidden layers: blockdiag3 weights [120,128pad] + tile3 bias [120,1]
    for l in range(15):
        b_, li_ = divmod(l, 3)
        W, b = blocks[b_][li_]
        d[f"hW{l}"] = _f16(_pad_cols(_blockdiag([W, W, W])))
        d[f"hB{l}"] = _f32(np.tile(b, 3)[:, None])
    # out-layer weights per group: [120, 3], col c = c_t(c,g) * w_out
    for g in range(3):
        Wo = np.zeros((120, 3), np.float32)
        for c in range(3):
            t = 3 * c + g
            ct = X_TICKS[t] * (1.0 - X_TICKS[t])
            Wo[c * 40:(c + 1) * 40, c] = ct * w_out
        d[f"Wog{g}"] = _f16(Wo)
    return d, w_out, b_out


def _prep_core(y_sh, z_sh, b_out):
    """Per-core input arrays. y_sh [1024,9] f32, z_sh [1024,8,5] f32."""
    d = {}
    yT = y_sh.T                                   # [9, 1024]
    d["y2T"] = _f16(np.concatenate([yT, yT], axis=0))   # [18, 1024]
    # zrep [15, 8192]: row r -> z[:, :, j_r] flattened n-major, p-minor
    j_of_row = np.concatenate([np.arange(D_XI), TJ])
    zr = z_sh[:, :, j_of_row].reshape(NXI, 15).T  # [15, 8192]
    d["zrep"] = _f16(zr)
    # D [3, 24576]: D[c, q] = y[n, t] - a_t - c_t*b_out, t=3c+q//8192, n=(q%8192)//8
    Dm = np.empty((3, 3 * NXI), np.float32)
    for c in range(3):
        for g in range(3):
            t = 3 * c + g
            a_t = B1 * (1.0 - X_TICKS[t]) + B2 * X_TICKS[t]
            c_t = X_TICKS[t] * (1.0 - X_TICKS[t])
            col = y_sh[:, t].repeat(P) - a_t - c_t * b_out      # [8192]
            Dm[c, g * NXI:(g + 1) * NXI] = col
    d["D"] = _f32(Dm)
    return d


_SHAPES = None


def _input_shapes(shared, core0):
    sh = {}
    for k, v in {**shared, **core0}.items():
        sh[k] = (tuple(v.shape), v.dtype)
    return sh


def _build_bass(shapes):
    import concourse.bacc as bacc
    import concourse.mybir as mybir
    import concourse.tile as tile

    F32 = mybir.dt.float32
    F16 = mybir.dt.float16
    AF = mybir.ActivationFunctionType
    ALU = mybir.AluOpType
    AX = mybir.AxisListType

    nc = bacc.Bacc("TRN2")
    dram = {}
    for name, (shape, dtype) in shapes.items():
        dt = F16 if dtype == np.float16 else F32
        dram[name] = nc.dram_tensor(name, shape, dt, kind="ExternalInput")
    out_d = nc.dram_tensor("out", (128, 32), F32, kind="ExternalOutput")

    with tile.TileContext(nc) as tc:
        with tc.tile_pool(name="const", bufs=1) as cpool, \
             tc.tile_pool(name="encsb", bufs=4) as encsb, \
             tc.tile_pool(name="hA", bufs=4) as hA, \
             tc.tile_pool(name="hB", bufs=4) as hB, \
             tc.tile_pool(name="epi", bufs=4) as epi, \
             tc.tile_pool(name="dsl", bufs=2) as dsl, \
             tc.tile_pool(name="psA", bufs=1, space="PSUM") as psA, \
             tc.tile_pool(name="psB", bufs=1, space="PSUM") as psB:

            # ---- load constants & inputs to SBUF ----
            sb = {}
            for name, (shape, dtype) in shapes.items():
                if name == "D":
                    continue  # stays in DRAM; sliced per supertile
                dt = F16 if dtype == np.float16 else F32
                t = cpool.tile(list(shape), dt, name=f"sb_{name}", tag=f"sb_{name}")
                nc.sync.dma_start(out=t[:], in_=dram[name][:])
                sb[name] = t

            accs = cpool.tile([128, 32], F32, name="accs", tag="accs")
            nc.vector.memset(accs[:], 0.0)

            # ---- encoder: mu trunk ----
            def enc_layer(rhs_t, rp, lhs_name, bias_name, op, out_p, out_name):
                """one encoder layer: 2 MMs of 512 cols + act/bias -> f16 tile"""
                ps = psA.tile([128, 2048], F32, name=f"ps_{out_name}", tag="psA")
                for k in range(2):
                    nc.tensor.matmul(ps[0:out_p, k * 512:(k + 1) * 512],
                                     sb[lhs_name][:],
                                     rhs_t[0:rp, k * 512:(k + 1) * 512],
                                     start=True, stop=True)
                o = encsb.tile([128, 1024], F16, name=out_name, tag="enc")
                if op == "relu":
                    nc.scalar.activation(o[0:out_p, :], ps[0:out_p, 0:1024], AF.Relu,
                                         bias=sb[bias_name][:], scale=1.0)
                else:  # linear + bias via DVE
                    nc.vector.tensor_scalar(out=o[0:out_p, :], in0=ps[0:out_p, 0:1024],
                                            scalar1=sb[bias_name][:], scalar2=None,
                                            op0=ALU.add)
                return o

            dims_m = [(9, 50), (50, 40), (40, 30), (30, 20), (20, 5)]
            cur = sb["y2T"]
            curp = 9
            for li, (kin, kout) in enumerate(dims_m):
                op = "relu" if li < 4 else "lin"
                cur = enc_layer(cur, curp, f"mW{li}", f"mB{li}", op, kout, f"mu_l{li}")
                curp = kout
            mu_sb = cur                                   # [5, 1024] f16

            dims_p = [(18, 100), (100, 80), (80, 60), (60, 40), (40, 15)]
            cur = sb["y2T"]
            curp = 18
            for li, (kin, kout) in enumerate(dims_p[:4]):
                cur = enc_layer(cur, curp, f"pW{li}", f"pB{li}", "relu", kout, f"pk_l{li}")
                curp = kout
            # final packed layer -> psum [15, 1024]; softplus diag, raw loff
            psf = psA.tile([128, 2048], F32, name="ps_pk4", tag="psA")
            for k in range(2):
                nc.tensor.matmul(psf[0:15, k * 512:(k + 1) * 512], sb["pW4"][:],
                                 cur[0:40, k * 512:(k + 1) * 512],
                                 start=True, stop=True)
            Lstack = cpool.tile([15, 1024], F16, name="Lstack", tag="Lstack")
            psfb = encsb.tile([128, 1024], F32, name="psfb", tag="enc32")
            nc.vector.tensor_scalar(out=psfb[0:15, :], in0=psf[0:15, 0:1024],
                                    scalar1=sb["pB4"][0:15, 0:1], scalar2=None,
                                    op0=ALU.add)
            nc.vector.tensor_copy(Lstack[0:15, :], psfb[0:15, :])
            exp_t = encsb.tile([128, 1024], F32, name="exp_t", tag="enc32b")
            nc.scalar.activation(exp_t[0:5, :], psfb[0:5, :], AF.Exp,
                                 bias=0.0, scale=1.0)
            nc.scalar.activation(Lstack[0:5, :], exp_t[0:5, :], AF.Ln,
                                 bias=1.0, scale=1.0)
            # entropy: sum ln(q_Ld) -> accs[0:5, 16]
            ent_dummy = encsb.tile([128, 1024], F16, name="ent_dummy", tag="enc")
            nc.scalar.activation(ent_dummy[0:5, :], Lstack[0:5, :], AF.Ln,
                                 bias=0.0, scale=1.0, accum_out=accs[0:5, 16:17])

            # ---- Mprod = Lstack (p-expanded) * zrep : [15, 8192] f16 ----
            Mprod = cpool.tile([15, 8192], F16, name="Mprod", tag="Mprod")
            z3 = sb["zrep"][:].rearrange("p (n q) -> p n q", q=8)
            L3 = Lstack[:, :, None].broadcast_to([15, 1024, 8])
            M3 = Mprod[:].rearrange("p (n q) -> p n q", q=8)
            nc.vector.tensor_tensor(out=M3, in0=z3, in1=L3, op=ALU.mult)

            def mu_bcast(w, k):
                """mu window chunk as [5, 64, 8] broadcast AP (512 cols)"""
                c0 = 256 * w + 64 * k
                return mu_sb[0:5, c0:c0 + 64, None].broadcast_to([5, 64, 8])

            # ---- xi^2 prior: 4 windows ----
            sq_dummy = cpool.tile([5, 2048], F16, name="sq_dummy", tag="sq_dummy")
            for w in range(NWIN):
                psx = psA.tile([128, 2048], F32, name=f"ps_xi{w}", tag="psA")
                for k in range(4):
                    cs = slice(k * 512, (k + 1) * 512)
                    nc.tensor.matmul(psx[0:5, cs], sb["SXI"][:],
                                     Mprod[:, 2048 * w + k * 512: 2048 * w + (k + 1) * 512],
                                     start=True, stop=False)
                    nc.tensor.matmul(psx[0:5, cs], sb["I5"][:], mu_bcast(w, k),
                                     start=False, stop=True)
                nc.scalar.activation(sq_dummy[:], psx[0:5, :], AF.Square,
                                     bias=0.0, scale=1.0,
                                     accum_out=accs[0:5, 12 + w:13 + w])

            # ---- main loop: 6 pairs of supertiles ----
            class St:
                pass

            def emit_h1(st):
                st.ps = st.pspool.tile([128, 2048], F32, name=f"ps{st.s}_h1",
                                       tag=st.pstag)
                w = st.s % 4
                for k in range(4):
                    cs = slice(k * 512, (k + 1) * 512)
                    nc.tensor.matmul(st.ps[:, cs], sb["AG"][:],
                                     Mprod[:, 2048 * w + k * 512: 2048 * w + (k + 1) * 512],
                                     start=True, stop=False)
                    nc.tensor.matmul(st.ps[:, cs], sb["BG"][:], mu_bcast(w, k),
                                     start=False, stop=True)
                h = st.hpool.tile([128, 2048], F16, name=f"h{st.s}_1", tag=st.htag)
                g = st.s // 4
                nc.scalar.activation(h[0:120, :], st.ps[0:120, :], AF.Silu,
                                     bias=sb[f"b1g{g}"][0:120, 0:1], scale=1.0)
                st.base = h
                st.r = h

            def emit_hidden(st, l):
                ps = st.pspool.tile([128, 2048], F32, name=f"ps{st.s}_l{l}",
                                    tag=st.pstag)
                for k in range(4):
                    cs = slice(k * 512, (k + 1) * 512)
                    nc.tensor.matmul(ps[:, cs], sb[f"hW{l}"][:], st.r[0:120, cs],
                                     start=True, stop=True)
                rn = st.hpool.tile([128, 2048], F16, name=f"h{st.s}_r{l}", tag=st.htag)
                nc.scalar.activation(rn[0:120, :], ps[0:120, :], AF.Silu,
                                     bias=sb[f"hB{l}"][0:120, 0:1], scale=1.0)
                st.r = rn
                if l % 3 == 2:   # block end: residual add
                    bn = st.hpool.tile([128, 2048], F16, name=f"h{st.s}_b{l}",
                                       tag=st.htag)
                    nc.vector.tensor_tensor(out=bn[0:120, :], in0=st.base[0:120, :],
                                            in1=rn[0:120, :], op=ALU.add)
                    st.base = bn
                    st.r = bn

            def emit_epilogue(st):
                g = st.s // 4
                pso = st.pspool.tile([128, 2048], F32, name=f"ps{st.s}_out",
                                     tag=st.pstag)
                for k in range(4):
                    cs = slice(k * 512, (k + 1) * 512)
                    nc.tensor.matmul(pso[0:3, cs], sb[f"Wog{g}"][:], st.r[0:120, cs],
                                     start=True, stop=True)
                Dsb = dsl.tile([3, 2048], F32, name=f"D{st.s}", tag="Dslice")
                nc.sync.dma_start(out=Dsb[:],
                                  in_=dram["D"][:, FD * st.s:FD * (st.s + 1)])
                dd = epi.tile([3, 2048], F16, name=f"d{st.s}", tag="epi_d")
                nc.vector.tensor_tensor(out=dd[:], in0=pso[0:3, :], in1=Dsb[:],
                                        op=ALU.subtract)
                dsq = epi.tile([3, 2048], F16, name=f"dsq{st.s}", tag="epi_q")
                nc.vector.tensor_tensor(out=dsq[:], in0=dd[:], in1=dd[:], op=ALU.mult)
                nc.vector.tensor_reduce(out=accs[0:3, st.s:st.s + 1], in_=dsq[:],
                                        axis=AX.X, op=ALU.add)

            for pair in range(6):
                sts = []
                for idx, s in enumerate((2 * pair, 2 * pair + 1)):
                    st = St()
                    st.s = s
                    st.pspool, st.pstag = (psA, "psA") if idx == 0 else (psB, "psB")
                    st.hpool, st.htag = (hA, "hA") if idx == 0 else (hB, "hB")
                    sts.append(st)
                for st in sts:
                    emit_h1(st)
                for l in range(15):
                    for st in sts:
                        emit_hidden(st, l)
                for st in sts:
                    emit_epilogue(st)

            nc.sync.dma_start(out=out_d[:], in_=accs[:])

    nc.compile()
    return nc


def kernel(y, z, mu_params, ldiag_params, loff_params, pinn_params):
    from concourse.bass_utils import run_bass_kernel_spmd

    y = _as_np(y)
    z = _as_np(z)
    shared, w_out, b_out = _prep_shared(mu_params, ldiag_params, loff_params,
                                        pinn_params)
    in_maps = []
    for i in range(NCORES):
        ysh = y[i * NLOC:(i + 1) * NLOC]
        zsh = z[i * NLOC:(i + 1) * NLOC]
        m = dict(shared)
        m.update(_prep_core(ysh, zsh, b_out))
        in_maps.append(m)

    shapes = _input_shapes(shared, _prep_core(y[:NLOC], z[:NLOC], b_out))
    nc = _build_bass(shapes)
    res = run_bass_kernel_spmd(nc, in_maps, core_ids=list(range(NCORES)))

    total = 0.0
    for i in range(NCORES):
        o = np.asarray(res.results[i]["out"], dtype=np.float64)
        Ssq = o[0:3, 0:12].sum()
        Sxi2 = o[0:5, 12:16].sum()
        Sent = o[0:5, 16].sum()
        core = (Sent + NLOC * 0.5 * D_XI * (1.0 + LOG2PI)
                + (1.0 / P) * (-0.5 * Sxi2 - 0.5 * D_XI * LOG2PI * NXI)
                + (1.0 / P) * (-(0.5 / NOISE ** 2) * Ssq
                               + NXI * N_TICKS * (-np.log(NOISE) - 0.5 * LOG2PI)))
        total += core
    loss = total / (NCORES * NLOC)
    return np.float32(loss)


# revision 4
# speedup vs baseline: 1.3054x; 1.3054x over previous
"""Amortized-VI loss kernel for 8 TRN2 NeuronCores (data-parallel).

Reference computation: 3 encoder MLPs on y -> (mu, Ldiag, Loff), Cholesky
reparameterization xi = mu + L z, a DenseResNet PINN evaluated at 9 x-ticks
per (sample, particle), Gaussian log-likelihood + prior + entropy, mean-reduced
to one scalar.

Device strategy (per core, n_loc=1024 samples):
  - feature-on-partition layout; 3-way block-diagonal weight packing so the
    128-wide engines see [120, N] tiles (3 chunks x 40 features).
  - PINN rows ordered tick-major: global row = t*8192 + (n*8+p). Chunk c
    covers ticks 3c..3c+2; 12 supertiles of 2048 columns; supertile s has
    tick-group g=s//4 and xi-window w=s%4, shared by all 3 chunks.
  - x-tick input folds into a per-(chunk,group) bias on layer 1; the output
    layer folds the likelihood residual: D = y - a_t - c_t*b_out precomputed
    on host, d = c_t*net_raw - D, accumulate sum(d^2).
  - loss reduces to three sums (resid^2, xi^2, ln softplus) finished on host.
  - ScalarE (silu) is the bottleneck: ACT ops read 4 PSUM banks at FD=2048.
    Two supertiles in flight (4+4 PSUM banks) keep ACT saturated; a PE spin
    warms the HAM clock gate before the main loop so matmuls run at 2.4 GHz.
"""

import numpy as np

D_XI = 5
N_TICKS = 9
NOISE = 0.015
LOG2PI = float(np.log(2.0 * np.pi))
X_TICKS = np.linspace(0.15, 0.85, N_TICKS).astype(np.float64)
B1, B2 = 1.0, 0.0

NCORES = 8
NLOC = 1024            # samples per core
P = 8                  # particles
NXI = NLOC * P         # 8192 xi rows per core
ST = 12                # supertiles per core
FD = 2048              # supertile free dim
TI, TJ = np.tril_indices(D_XI, -1)   # strict-lower pairs (reference order)
NWIN = 4               # xi windows
WARMUP_MM = 28         # PE spin matmuls before the main loop


def _as_np(x):
    return np.asarray(x, dtype=np.float32)


def _blockdiag(mats):
    rows = sum(m.shape[0] for m in mats)
    cols = sum(m.shape[1] for m in mats)
    out = np.zeros((rows, cols), np.float32)
    r = c = 0
    for m in mats:
        out[r:r + m.shape[0], c:c + m.shape[1]] = m
        r += m.shape[0]
        c += m.shape[1]
    return out


class _Pack:
    """Concatenate [rows, cols] matrices along the free dim of one tensor."""

    def __init__(self):
        self.mats = {}
        self.off = {}
        self.cursor = 0

    def add(self, name, mat):
        self.off[name] = self.cursor
        self.mats[name] = np.asarray(mat, np.float32)
        self.cursor += mat.shape[1]

    def build(self, dtype):
        buf = np.zeros((128, self.cursor), dtype)
        for name, m in self.mats.items():
            buf[:m.shape[0], self.off[name]:self.off[name] + m.shape[1]] = m
        return np.ascontiguousarray(buf)

    def view(self, tile_ap, name):
        m = self.mats[name]
        o = self.off[name]
        return tile_ap[0:m.shape[0], o:o + m.shape[1]]


def _prep_shared(mu_params, ldiag_params, loff_params, pinn_params):
    """Host-packed weights shared by all cores."""
    mu_p = [(_as_np(W), _as_np(b)) for (W, b) in mu_params]
    ld_p = [(_as_np(W), _as_np(b)) for (W, b) in ldiag_params]
    lo_p = [(_as_np(W), _as_np(b)) for (W, b) in loff_params]
    W_in, b_in = _as_np(pinn_params["inp"][0]), _as_np(pinn_params["inp"][1])
    blocks = [[(_as_np(W), _as_np(b)) for (W, b) in blk]
              for blk in pinn_params["blocks"]]
    W_out, b_out = _as_np(pinn_params["out"][0]), _as_np(pinn_params["out"][1])
    w_out = W_out[:, 0]
    b_out = float(b_out[0])

    wp = _Pack()   # float16 weights
    bp = _Pack()   # float32 per-partition bias columns
    for li, (W, b) in enumerate(mu_p):
        wp.add(f"mW{li}", W)
        bp.add(f"mB{li}", b[:, None])
    for li in range(5):
        wp.add(f"pW{li}", _blockdiag([ld_p[li][0], lo_p[li][0]]))
        bp.add(f"pB{li}", np.concatenate([ld_p[li][1], lo_p[li][1]])[:, None])
    W1 = W_in[1:, :]
    i_of_row = np.concatenate([np.arange(D_XI), TI])
    A = np.zeros((15, 128), np.float32)
    Bm = np.zeros((5, 128), np.float32)
    for c in range(3):
        for r in range(15):
            A[r, c * 40:(c + 1) * 40] = W1[i_of_row[r]]
        Bm[:, c * 40:(c + 1) * 40] = W1
    wp.add("AG", A)
    wp.add("BG", Bm)
    Sxi = np.zeros((15, D_XI), np.float32)
    for r in range(15):
        Sxi[r, i_of_row[r]] = 1.0
    wp.add("SXI", Sxi)
    wp.add("I5", np.eye(D_XI, dtype=np.float32))
    for g in range(3):
        bb = np.zeros((120, 1), np.float32)
        for c in range(3):
            t = 3 * c + g
            bb[c * 40:(c + 1) * 40, 0] = b_in + W_in[0, :] * X_TICKS[t]
        bp.add(f"b1g{g}", bb)
    for l in range(15):
        b_, li_ = divmod(l, 3)
        W, b = blocks[b_][li_]
        w3 = np.zeros((120, 128), np.float32)
        w3[:, :120] = _blockdiag([W, W, W])
        wp.add(f"hW{l}", w3)
        bp.add(f"hB{l}", np.tile(b, 3)[:, None])
    for g in range(3):
        Wo = np.zeros((120, 3), np.float32)
        for c in range(3):
            t = 3 * c + g
            ct = X_TICKS[t] * (1.0 - X_TICKS[t])
            Wo[c * 40:(c + 1) * 40, c] = ct * w_out
        wp.add(f"Wog{g}", Wo)

    shared = {"wpack": wp.build(np.float16), "bpack": bp.build(np.float32)}
    return shared, wp, bp, w_out, b_out


def _prep_core(y_sh, z_sh, b_out):
    """Per-core inputs. y_sh [1024,9] f32, z_sh [1024,8,5] f32."""
    d = {}
    yT = y_sh.T
    d["y2T"] = np.ascontiguousarray(
        np.concatenate([yT, yT], axis=0).astype(np.float16))
    j_of_row = np.concatenate([np.arange(D_XI), TJ])
    zr = z_sh[:, :, j_of_row].reshape(NXI, 15).T
    d["zrep"] = np.ascontiguousarray(zr.astype(np.float16))
    Dm = np.empty((3, 3 * NXI), np.float32)
    for c in range(3):
        for g in range(3):
            t = 3 * c + g
            a_t = B1 * (1.0 - X_TICKS[t]) + B2 * X_TICKS[t]
            c_t = X_TICKS[t] * (1.0 - X_TICKS[t])
            Dm[c, g * NXI:(g + 1) * NXI] = y_sh[:, t].repeat(P) - a_t - c_t * b_out
    d["D"] = np.ascontiguousarray(Dm)
    return d


def _build_bass(shapes, wp, bp):
    import concourse.bacc as bacc
    import concourse.mybir as mybir
    import concourse.tile as tile

    F32 = mybir.dt.float32
    F16 = mybir.dt.float16
    AF = mybir.ActivationFunctionType
    ALU = mybir.AluOpType
    AX = mybir.AxisListType

    nc = bacc.Bacc("TRN2")
    dram = {}
    for name, (shape, dtype) in shapes.items():
        dt = F16 if dtype == np.float16 else F32
        dram[name] = nc.dram_tensor(name, shape, dt, kind="ExternalInput")
    out_d = nc.dram_tensor("out", (128, 32), F32, kind="ExternalOutput")

    with tile.TileContext(nc) as tc:
        with tc.tile_pool(name="const", bufs=1) as cpool, \
             tc.tile_pool(name="encsb", bufs=4) as encsb, \
             tc.tile_pool(name="hA", bufs=4) as hA, \
             tc.tile_pool(name="hB", bufs=4) as hB, \
             tc.tile_pool(name="epi", bufs=4) as epi, \
             tc.tile_pool(name="dsl", bufs=3) as dsl, \
             tc.tile_pool(name="psA", bufs=1, space="PSUM") as psA, \
             tc.tile_pool(name="psB", bufs=1, space="PSUM") as psB:

            wpack = cpool.tile(list(shapes["wpack"][0]), F16, name="wpack",
                               tag="wpack")
            bpack = cpool.tile(list(shapes["bpack"][0]), F32, name="bpack",
                               tag="bpack")
            y2T = cpool.tile([18, 1024], F16, name="y2T", tag="y2T")
            zrep = cpool.tile([15, NXI], F16, name="zrep", tag="zrep")
            nc.sync.dma_start(out=wpack[:], in_=dram["wpack"][:])
            nc.sync.dma_start(out=bpack[:], in_=dram["bpack"][:])
            nc.sync.dma_start(out=y2T[:], in_=dram["y2T"][:])
            nc.sync.dma_start(out=zrep[:], in_=dram["zrep"][:])

            def W(name):
                return wp.view(wpack, name)

            def B(name, p=None):
                v = bp.view(bpack, name)
                return v if p is None else v[0:p, 0:1]

            accs = cpool.tile([128, 32], F32, name="accs", tag="accs")
            nc.vector.memset(accs[:], 0.0)

            # ---- encoder ----
            def enc_layer(rhs_t, rp, lhs, bias, op, out_p, out_name):
                ps = psA.tile([128, 2048], F32, name=f"ps_{out_name}", tag="psA")
                for k in range(2):
                    nc.tensor.matmul(ps[0:out_p, k * 512:(k + 1) * 512], lhs,
                                     rhs_t[0:rp, k * 512:(k + 1) * 512],
                                     start=True, stop=True)
                o = encsb.tile([128, 1024], F16, name=out_name, tag="enc")
                if op == "relu":
                    nc.scalar.activation(o[0:out_p, :], ps[0:out_p, 0:1024],
                                         AF.Relu, bias=bias, scale=1.0)
                else:
                    nc.vector.tensor_scalar(out=o[0:out_p, :],
                                            in0=ps[0:out_p, 0:1024],
                                            scalar1=bias, scalar2=None,
                                            op0=ALU.add)
                return o

            dims_m = [(9, 50), (50, 40), (40, 30), (30, 20), (20, 5)]
            cur, curp = y2T, 9
            for li, (kin, kout) in enumerate(dims_m):
                op = "relu" if li < 4 else "lin"
                cur = enc_layer(cur, curp, W(f"mW{li}"), B(f"mB{li}", kout), op,
                                kout, f"mu_l{li}")
                curp = kout
            mu_sb = cur

            dims_p = [(18, 100), (100, 80), (80, 60), (60, 40)]
            cur, curp = y2T, 18
            for li, (kin, kout) in enumerate(dims_p):
                cur = enc_layer(cur, curp, W(f"pW{li}"), B(f"pB{li}", kout),
                                "relu", kout, f"pk_l{li}")
                curp = kout
            psf = psA.tile([128, 2048], F32, name="ps_pk4", tag="psA")
            for k in range(2):
                nc.tensor.matmul(psf[0:15, k * 512:(k + 1) * 512], W("pW4"),
                                 cur[0:40, k * 512:(k + 1) * 512],
                                 start=True, stop=True)
            Lstack = cpool.tile([15, 1024], F16, name="Lstack", tag="Lstack")
            psfb = encsb.tile([128, 1024], F32, name="psfb", tag="enc32")
            nc.vector.tensor_scalar(out=psfb[0:15, :], in0=psf[0:15, 0:1024],
                                    scalar1=B("pB4", 15), scalar2=None,
                                    op0=ALU.add)
            nc.vector.tensor_copy(Lstack[0:15, :], psfb[0:15, :])
            exp_t = encsb.tile([128, 1024], F32, name="exp_t", tag="enc32b")
            nc.scalar.activation(exp_t[0:5, :], psfb[0:5, :], AF.Exp,
                                 bias=0.0, scale=1.0)
            nc.scalar.activation(Lstack[0:5, :], exp_t[0:5, :], AF.Ln,
                                 bias=1.0, scale=1.0)
            ent_dummy = encsb.tile([128, 1024], F16, name="ent_dummy", tag="enc")
            nc.scalar.activation(ent_dummy[0:5, :], Lstack[0:5, :], AF.Ln,
                                 bias=0.0, scale=1.0, accum_out=accs[0:5, 16:17])

            # ---- Mprod = (p-expanded Lstack) * zrep ----
            Mprod = cpool.tile([15, NXI], F16, name="Mprod", tag="Mprod")
            z3 = zrep[:].rearrange("p (n q) -> p n q", q=8)
            L3 = Lstack[:, :, None].broadcast_to([15, 1024, 8])
            M3 = Mprod[:].rearrange("p (n q) -> p n q", q=8)
            nc.vector.tensor_tensor(out=M3, in0=z3, in1=L3, op=ALU.mult)

            def mu_bcast(w, k):
                c0 = 256 * w + 64 * k
                return mu_sb[0:5, c0:c0 + 64, None].broadcast_to([5, 64, 8])

            # ---- xi^2 prior over 4 windows ----
            sq_dummy = cpool.tile([5, 2048], F16, name="sq_dummy", tag="sq_dummy")
            for w in range(NWIN):
                psx = psA.tile([128, 2048], F32, name=f"ps_xi{w}", tag="psA")
                for k in range(4):
                    cs = slice(k * 512, (k + 1) * 512)
                    nc.tensor.matmul(psx[0:5, cs], W("SXI"),
                                     Mprod[:, FD * w + k * 512: FD * w + (k + 1) * 512],
                                     start=True, stop=False)
                    nc.tensor.matmul(psx[0:5, cs], W("I5"), mu_bcast(w, k),
                                     start=False, stop=True)
                nc.scalar.activation(sq_dummy[:], psx[0:5, :], AF.Square,
                                     bias=0.0, scale=1.0,
                                     accum_out=accs[0:5, 12 + w:13 + w])

            # ---- PE warmup spin: flips the HAM clock gate to 2.4 GHz ----
            ps_spin = psB.tile([128, 2048], F32, name="ps_spin", tag="psB")
            for i in range(WARMUP_MM):
                nc.tensor.matmul(ps_spin[:, (i % 4) * 512:(i % 4) * 512 + 512],
                                 W("hW0"), wpack[0:120, 0:512],
                                 start=True, stop=True)

            # ---- main loop: 6 pairs of supertiles ----
            class St:
                pass

            def emit_h1(st):
                Dsb = dsl.tile([3, 2048], F32, name=f"D{st.s}", tag="Dslice")
                nc.sync.dma_start(out=Dsb[:],
                                  in_=dram["D"][:, FD * st.s:FD * (st.s + 1)])
                st.Dsb = Dsb
                st.ps = st.pspool.tile([128, 2048], F32, name=f"ps{st.s}_h1",
                                       tag=st.pstag)
                w = st.s % 4
                for k in range(4):
                    cs = slice(k * 512, (k + 1) * 512)
                    nc.tensor.matmul(st.ps[:, cs], W("AG"),
                                     Mprod[:, FD * w + k * 512: FD * w + (k + 1) * 512],
                                     start=True, stop=False)
                    nc.tensor.matmul(st.ps[:, cs], W("BG"), mu_bcast(w, k),
                                     start=False, stop=True)
                h = st.hpool.tile([128, 2048], F16, name=f"h{st.s}_1", tag=st.htag)
                g = st.s // 4
                nc.scalar.activation(h[0:120, :], st.ps[0:120, :], AF.Silu,
                                     bias=B(f"b1g{g}", 120), scale=1.0)
                st.base = h
                st.r = h

            def emit_hidden(st, l):
                ps = st.pspool.tile([128, 2048], F32, name=f"ps{st.s}_l{l}",
                                    tag=st.pstag)
                for k in range(4):
                    cs = slice(k * 512, (k + 1) * 512)
                    nc.tensor.matmul(ps[:, cs], W(f"hW{l}"), st.r[0:120, cs],
                                     start=True, stop=True)
                rn = st.hpool.tile([128, 2048], F16, name=f"h{st.s}_r{l}",
                                   tag=st.htag)
                nc.scalar.activation(rn[0:120, :], ps[0:120, :], AF.Silu,
                                     bias=B(f"hB{l}", 120), scale=1.0)
                st.r = rn
                if l % 3 == 2:
                    bn = st.hpool.tile([128, 2048], F16, name=f"h{st.s}_b{l}",
                                       tag=st.htag)
                    nc.vector.tensor_tensor(out=bn[0:120, :], in0=st.base[0:120, :],
                                            in1=rn[0:120, :], op=ALU.add)
                    st.base = bn
                    st.r = bn

            def emit_epilogue(st):
                g = st.s // 4
                pso = st.pspool.tile([128, 2048], F32, name=f"ps{st.s}_out",
                                     tag=st.pstag)
                for k in range(4):
                    cs = slice(k * 512, (k + 1) * 512)
                    nc.tensor.matmul(pso[0:3, cs], W(f"Wog{g}"), st.r[0:120, cs],
                                     start=True, stop=True)
                dd = epi.tile([3, 2048], F16, name=f"d{st.s}", tag="epi_d")
                nc.vector.tensor_tensor(out=dd[:], in0=pso[0:3, :], in1=st.Dsb[:],
                                        op=ALU.subtract)
                dsq = epi.tile([3, 2048], F16, name=f"dsq{st.s}", tag="epi_q")
                nc.vector.tensor_tensor(out=dsq[:], in0=dd[:], in1=dd[:],
                                        op=ALU.mult)
                nc.vector.tensor_reduce(out=accs[0:3, st.s:st.s + 1], in_=dsq[:],
                                        axis=AX.X, op=ALU.add)

            for pair in range(6):
                sts = []
                for idx, s in enumerate((2 * pair, 2 * pair + 1)):
                    st = St()
                    st.s = s
                    st.pspool, st.pstag = (psA, "psA") if idx == 0 else (psB, "psB")
                    st.hpool, st.htag = (hA, "hA") if idx == 0 else (hB, "hB")
                    sts.append(st)
                for st in sts:
                    emit_h1(st)
                for l in range(15):
                    for st in sts:
                        emit_hidden(st, l)
                for st in sts:
                    emit_epilogue(st)

            nc.sync.dma_start(out=out_d[:], in_=accs[:])

    nc.compile()
    return nc


_CACHE = {}


def _get_nc(shapes, wp, bp):
    key = tuple(sorted((k, v[0], str(v[1])) for k, v in shapes.items()))
    if key not in _CACHE:
        _CACHE[key] = _build_bass(shapes, wp, bp)
    return _CACHE[key]


def kernel(y, z, mu_params, ldiag_params, loff_params, pinn_params):
    from concourse.bass_utils import run_bass_kernel_spmd

    y = _as_np(y)
    z = _as_np(z)
    shared, wp, bp, w_out, b_out = _prep_shared(mu_params, ldiag_params,
                                                loff_params, pinn_params)
    in_maps = []
    for i in range(NCORES):
        m = dict(shared)
        m.update(_prep_core(y[i * NLOC:(i + 1) * NLOC],
                            z[i * NLOC:(i + 1) * NLOC], b_out))
        in_maps.append(m)

    shapes = {k: (tuple(v.shape), v.dtype) for k, v in in_maps[0].items()}
    nc = _get_nc(shapes, wp, bp)
    res = run_bass_kernel_spmd(nc, in_maps, core_ids=list(range(NCORES)))

    total = 0.0
    for i in range(NCORES):
        o = np.asarray(res.results[i]["out"], dtype=np.float64)
        Ssq = o[0:3, 0:12].sum()
        Sxi2 = o[0:5, 12:16].sum()
        Sent = o[0:5, 16].sum()
        core = (Sent + NLOC * 0.5 * D_XI * (1.0 + LOG2PI)
                + (1.0 / P) * (-0.5 * Sxi2 - 0.5 * D_XI * LOG2PI * NXI)
                + (1.0 / P) * (-(0.5 / NOISE ** 2) * Ssq
                               + NXI * N_TICKS * (-np.log(NOISE) - 0.5 * LOG2PI)))
        total += core
    loss = total / (NCORES * NLOC)
    return np.float32(loss)


# revision 8
# speedup vs baseline: 1.3098x; 1.0034x over previous
"""Amortized-VI loss kernel for 8 TRN2 NeuronCores (data-parallel).

Reference computation: 3 encoder MLPs on y -> (mu, Ldiag, Loff), Cholesky
reparameterization xi = mu + L z, a DenseResNet PINN evaluated at 9 x-ticks
per (sample, particle), Gaussian log-likelihood + prior + entropy, mean-reduced
to one scalar.

Device strategy (per core, n_loc=1024 samples):
  - feature-on-partition layout; 3-way block-diagonal weight packing so the
    128-wide engines see [120, N] tiles (3 chunks x 40 features).
  - PINN rows ordered tick-major: global row = t*8192 + (n*8+p). Chunk c
    covers ticks 3c..3c+2; 12 supertiles of 2048 columns; supertile s has
    tick-group g=s//4 and xi-window w=s%4, shared by all 3 chunks.
  - x-tick input folds into a per-(chunk,group) bias on layer 1; the output
    layer folds the likelihood residual: D = y - a_t - c_t*b_out precomputed
    on host, d = c_t*net_raw - D, accumulate sum(d^2).
  - loss reduces to three sums (resid^2, xi^2, ln softplus) finished on host.
  - ScalarE (silu) is the bottleneck: ACT ops read 4 PSUM banks at FD=2048.
    Two supertiles in flight (4+4 PSUM banks) keep ACT saturated; a PE spin
    warms the HAM clock gate before the main loop so matmuls run at 2.4 GHz.
"""

import numpy as np

D_XI = 5
N_TICKS = 9
NOISE = 0.015
LOG2PI = float(np.log(2.0 * np.pi))
X_TICKS = np.linspace(0.15, 0.85, N_TICKS).astype(np.float64)
B1, B2 = 1.0, 0.0

NCORES = 8
NLOC = 1024            # samples per core
P = 8                  # particles
NXI = NLOC * P         # 8192 xi rows per core
ST = 12                # supertiles per core
FD = 2048              # supertile free dim
TI, TJ = np.tril_indices(D_XI, -1)   # strict-lower pairs (reference order)
NWIN = 4               # xi windows
WARMUP_MM = 28         # PE spin matmuls before the main loop


def _as_np(x):
    return np.asarray(x, dtype=np.float32)


def _blockdiag(mats):
    rows = sum(m.shape[0] for m in mats)
    cols = sum(m.shape[1] for m in mats)
    out = np.zeros((rows, cols), np.float32)
    r = c = 0
    for m in mats:
        out[r:r + m.shape[0], c:c + m.shape[1]] = m
        r += m.shape[0]
        c += m.shape[1]
    return out


class _Pack:
    """Concatenate [rows, cols] matrices along the free dim of one tensor."""

    def __init__(self):
        self.mats = {}
        self.off = {}
        self.cursor = 0

    def add(self, name, mat):
        self.off[name] = self.cursor
        self.mats[name] = np.asarray(mat, np.float32)
        self.cursor += mat.shape[1]

    def build(self, dtype):
        buf = np.zeros((128, self.cursor), dtype)
        for name, m in self.mats.items():
            buf[:m.shape[0], self.off[name]:self.off[name] + m.shape[1]] = m
        return np.ascontiguousarray(buf)

    def view(self, tile_ap, name):
        m = self.mats[name]
        o = self.off[name]
        return tile_ap[0:m.shape[0], o:o + m.shape[1]]


def _prep_shared(mu_params, ldiag_params, loff_params, pinn_params):
    """Host-packed weights shared by all cores."""
    mu_p = [(_as_np(W), _as_np(b)) for (W, b) in mu_params]
    ld_p = [(_as_np(W), _as_np(b)) for (W, b) in ldiag_params]
    lo_p = [(_as_np(W), _as_np(b)) for (W, b) in loff_params]
    W_in, b_in = _as_np(pinn_params["inp"][0]), _as_np(pinn_params["inp"][1])
    blocks = [[(_as_np(W), _as_np(b)) for (W, b) in blk]
              for blk in pinn_params["blocks"]]
    W_out, b_out = _as_np(pinn_params["out"][0]), _as_np(pinn_params["out"][1])
    w_out = W_out[:, 0]
    b_out = float(b_out[0])

    wp = _Pack()   # float16 weights
    bp = _Pack()   # float32 per-partition bias columns
    for li, (W, b) in enumerate(mu_p):
        wp.add(f"mW{li}", W)
        bp.add(f"mB{li}", b[:, None])
    for li in range(5):
        wp.add(f"pW{li}", _blockdiag([ld_p[li][0], lo_p[li][0]]))
        bp.add(f"pB{li}", np.concatenate([ld_p[li][1], lo_p[li][1]])[:, None])
    W1 = W_in[1:, :]
    i_of_row = np.concatenate([np.arange(D_XI), TI])
    A = np.zeros((15, 128), np.float32)
    Bm = np.zeros((5, 128), np.float32)
    for c in range(3):
        for r in range(15):
            A[r, c * 40:(c + 1) * 40] = W1[i_of_row[r]]
        Bm[:, c * 40:(c + 1) * 40] = W1
    wp.add("AG", A)
    wp.add("BG", Bm)
    Sxi = np.zeros((15, D_XI), np.float32)
    for r in range(15):
        Sxi[r, i_of_row[r]] = 1.0
    wp.add("SXI", Sxi)
    wp.add("I5", np.eye(D_XI, dtype=np.float32))
    for g in range(3):
        bb = np.zeros((120, 1), np.float32)
        for c in range(3):
            t = 3 * c + g
            bb[c * 40:(c + 1) * 40, 0] = b_in + W_in[0, :] * X_TICKS[t]
        bp.add(f"b1g{g}", bb)
    for l in range(15):
        b_, li_ = divmod(l, 3)
        W, b = blocks[b_][li_]
        w3 = np.zeros((120, 128), np.float32)
        w3[:, :120] = _blockdiag([W, W, W])
        wp.add(f"hW{l}", w3)
        bp.add(f"hB{l}", np.tile(b, 3)[:, None])
    for g in range(3):
        Wo = np.zeros((120, 3), np.float32)
        for c in range(3):
            t = 3 * c + g
            ct = X_TICKS[t] * (1.0 - X_TICKS[t])
            Wo[c * 40:(c + 1) * 40, c] = ct * w_out
        wp.add(f"Wog{g}", Wo)

    shared = {"wpack": wp.build(np.float16), "bpack": bp.build(np.float32)}
    return shared, wp, bp, w_out, b_out


def _prep_core(y_sh, z_sh, b_out):
    """Per-core inputs. y_sh [1024,9] f32, z_sh [1024,8,5] f32."""
    d = {}
    yT = y_sh.T
    d["y2T"] = np.ascontiguousarray(
        np.concatenate([yT, yT], axis=0).astype(np.float16))
    j_of_row = np.concatenate([np.arange(D_XI), TJ])
    zr = z_sh[:, :, j_of_row].reshape(NXI, 15).T
    d["zrep"] = np.ascontiguousarray(zr.astype(np.float16))
    Dm = np.empty((3, 3 * NXI), np.float32)
    for c in range(3):
        for g in range(3):
            t = 3 * c + g
            a_t = B1 * (1.0 - X_TICKS[t]) + B2 * X_TICKS[t]
            c_t = X_TICKS[t] * (1.0 - X_TICKS[t])
            Dm[c, g * NXI:(g + 1) * NXI] = y_sh[:, t].repeat(P) - a_t - c_t * b_out
    d["D"] = np.ascontiguousarray(Dm)
    return d


def _build_bass(shapes, wp, bp):
    import concourse.bacc as bacc
    import concourse.mybir as mybir
    import concourse.tile as tile

    F32 = mybir.dt.float32
    F16 = mybir.dt.float16
    AF = mybir.ActivationFunctionType
    ALU = mybir.AluOpType
    AX = mybir.AxisListType

    nc = bacc.Bacc("TRN2")
    dram = {}
    for name, (shape, dtype) in shapes.items():
        dt = F16 if dtype == np.float16 else F32
        dram[name] = nc.dram_tensor(name, shape, dt, kind="ExternalInput")
    out_d = nc.dram_tensor("out", (128, 32), F32, kind="ExternalOutput")

    with tile.TileContext(nc) as tc:
        with tc.tile_pool(name="const", bufs=1) as cpool, \
             tc.tile_pool(name="encsb", bufs=4) as encsb, \
             tc.tile_pool(name="hA", bufs=4) as hA, \
             tc.tile_pool(name="hB", bufs=4) as hB, \
             tc.tile_pool(name="epi", bufs=4) as epi, \
             tc.tile_pool(name="dsl", bufs=3) as dsl, \
             tc.tile_pool(name="psA", bufs=1, space="PSUM") as psA, \
             tc.tile_pool(name="psB", bufs=1, space="PSUM") as psB:

            wpack = cpool.tile(list(shapes["wpack"][0]), F16, name="wpack",
                               tag="wpack")
            bpack = cpool.tile(list(shapes["bpack"][0]), F32, name="bpack",
                               tag="bpack")
            y2T = cpool.tile([18, 1024], F16, name="y2T", tag="y2T")
            zrep = cpool.tile([15, NXI], F16, name="zrep", tag="zrep")
            nc.sync.dma_start(out=wpack[:], in_=dram["wpack"][:])
            nc.sync.dma_start(out=bpack[:], in_=dram["bpack"][:])
            nc.sync.dma_start(out=y2T[:], in_=dram["y2T"][:])
            nc.sync.dma_start(out=zrep[:], in_=dram["zrep"][:])

            def W(name):
                return wp.view(wpack, name)

            def B(name, p=None):
                v = bp.view(bpack, name)
                return v if p is None else v[0:p, 0:1]

            accs = cpool.tile([128, 32], F32, name="accs", tag="accs")
            nc.vector.memset(accs[:], 0.0)

            # ---- encoder ----
            def enc_layer(rhs_t, rp, lhs, bias, op, out_p, out_name):
                ps = psA.tile([128, 2048], F32, name=f"ps_{out_name}", tag="psA")
                for k in range(2):
                    nc.tensor.matmul(ps[0:out_p, k * 512:(k + 1) * 512], lhs,
                                     rhs_t[0:rp, k * 512:(k + 1) * 512],
                                     start=True, stop=True)
                o = encsb.tile([128, 1024], F16, name=out_name, tag="enc")
                if op == "relu":
                    nc.scalar.activation(o[0:out_p, :], ps[0:out_p, 0:1024],
                                         AF.Relu, bias=bias, scale=1.0)
                else:
                    nc.vector.tensor_scalar(out=o[0:out_p, :],
                                            in0=ps[0:out_p, 0:1024],
                                            scalar1=bias, scalar2=None,
                                            op0=ALU.add)
                return o

            # PE warmup spin interleaved with the encoder chain: keeps the
            # TensorE HAM clock gate busy so matmuls run at 2.4 GHz.
            ps_spin = psB.tile([128, 2048], F32, name="ps_spin", tag="psB")
            spin_left = [WARMUP_MM]

            def spin(nmm):
                nmm = min(nmm, spin_left[0])
                spin_left[0] -= nmm
                for i in range(nmm):
                    nc.tensor.matmul(
                        ps_spin[:, (i % 4) * 512:(i % 4) * 512 + 512],
                        W("hW0"), wpack[0:120, 0:512], start=True, stop=True)

            dims_m = [(9, 50), (50, 40), (40, 30), (30, 20), (20, 5)]
            cur, curp = y2T, 9
            for li, (kin, kout) in enumerate(dims_m):
                op = "relu" if li < 4 else "lin"
                cur = enc_layer(cur, curp, W(f"mW{li}"), B(f"mB{li}", kout), op,
                                kout, f"mu_l{li}")
                curp = kout
                spin(3)
            mu_sb = cur

            dims_p = [(18, 100), (100, 80), (80, 60), (60, 40)]
            cur, curp = y2T, 18
            for li, (kin, kout) in enumerate(dims_p):
                cur = enc_layer(cur, curp, W(f"pW{li}"), B(f"pB{li}", kout),
                                "relu", kout, f"pk_l{li}")
                curp = kout
                spin(3)
            psf = psA.tile([128, 2048], F32, name="ps_pk4", tag="psA")
            for k in range(2):
                nc.tensor.matmul(psf[0:15, k * 512:(k + 1) * 512], W("pW4"),
                                 cur[0:40, k * 512:(k + 1) * 512],
                                 start=True, stop=True)
            Lstack = cpool.tile([15, 1024], F16, name="Lstack", tag="Lstack")
            psfb = encsb.tile([128, 1024], F32, name="psfb", tag="enc32")
            nc.vector.tensor_scalar(out=psfb[0:15, :], in0=psf[0:15, 0:1024],
                                    scalar1=B("pB4", 15), scalar2=None,
                                    op0=ALU.add)
            nc.vector.tensor_copy(Lstack[0:15, :], psfb[0:15, :])
            exp_t = encsb.tile([128, 1024], F32, name="exp_t", tag="enc32b")
            nc.scalar.activation(exp_t[0:5, :], psfb[0:5, :], AF.Exp,
                                 bias=0.0, scale=1.0)
            nc.scalar.activation(Lstack[0:5, :], exp_t[0:5, :], AF.Ln,
                                 bias=1.0, scale=1.0)
            ent_dummy = encsb.tile([128, 1024], F16, name="ent_dummy", tag="enc")
            nc.scalar.activation(ent_dummy[0:5, :], Lstack[0:5, :], AF.Ln,
                                 bias=0.0, scale=1.0, accum_out=accs[0:5, 16:17])

            # ---- Mprod = (p-expanded Lstack) * zrep, in 4 window chunks ----
            Mprod = cpool.tile([15, NXI], F16, name="Mprod", tag="Mprod")
            for w in range(NWIN):
                n0 = 256 * w
                z3 = zrep[:, FD * w:FD * (w + 1)].rearrange(
                    "p (n q) -> p n q", q=8)
                L3 = Lstack[:, n0:n0 + 256, None].broadcast_to([15, 256, 8])
                M3 = Mprod[:, FD * w:FD * (w + 1)].rearrange(
                    "p (n q) -> p n q", q=8)
                nc.vector.tensor_tensor(out=M3, in0=z3, in1=L3, op=ALU.mult)

            def mu_bcast(w, k):
                c0 = 256 * w + 64 * k
                return mu_sb[0:5, c0:c0 + 64, None].broadcast_to([5, 64, 8])

            # ---- xi^2 prior over 4 windows ----
            sq_dummy = cpool.tile([5, 2048], F16, name="sq_dummy", tag="sq_dummy")
            for w in range(NWIN):
                psx = psA.tile([128, 2048], F32, name=f"ps_xi{w}", tag="psA")
                for k in range(4):
                    cs = slice(k * 512, (k + 1) * 512)
                    nc.tensor.matmul(psx[0:5, cs], W("SXI"),
                                     Mprod[:, FD * w + k * 512: FD * w + (k + 1) * 512],
                                     start=True, stop=False)
                    nc.tensor.matmul(psx[0:5, cs], W("I5"), mu_bcast(w, k),
                                     start=False, stop=True)
                nc.scalar.activation(sq_dummy[:], psx[0:5, :], AF.Square,
                                     bias=0.0, scale=1.0,
                                     accum_out=accs[0:5, 12 + w:13 + w])

            spin(spin_left[0])

            # ---- main loop: two software-pipelined lanes, 9-step stagger ----
            class St:
                pass

            def emit_h1(st):
                Dsb = dsl.tile([3, 2048], F32, name=f"D{st.s}", tag="Dslice")
                nc.sync.dma_start(out=Dsb[:],
                                  in_=dram["D"][:, FD * st.s:FD * (st.s + 1)])
                st.Dsb = Dsb
                st.ps = st.pspool.tile([128, 2048], F32, name=f"ps{st.s}_h1",
                                       tag=st.pstag)
                w = st.s % 4
                for k in range(4):
                    cs = slice(k * 512, (k + 1) * 512)
                    nc.tensor.matmul(st.ps[:, cs], W("AG"),
                                     Mprod[:, FD * w + k * 512: FD * w + (k + 1) * 512],
                                     start=True, stop=False)
                    nc.tensor.matmul(st.ps[:, cs], W("BG"), mu_bcast(w, k),
                                     start=False, stop=True)
                h = st.hpool.tile([128, 2048], F16, name=f"h{st.s}_1", tag=st.htag)
                g = st.s // 4
                nc.scalar.activation(h[0:120, :], st.ps[0:120, :], AF.Silu,
                                     bias=B(f"b1g{g}", 120), scale=1.0)
                st.base = h
                st.r = h

            def emit_hidden(st, l):
                ps = st.pspool.tile([128, 2048], F32, name=f"ps{st.s}_l{l}",
                                    tag=st.pstag)
                for k in range(4):
                    cs = slice(k * 512, (k + 1) * 512)
                    nc.tensor.matmul(ps[:, cs], W(f"hW{l}"), st.r[0:120, cs],
                                     start=True, stop=True)
                rn = st.hpool.tile([128, 2048], F16, name=f"h{st.s}_r{l}",
                                   tag=st.htag)
                nc.scalar.activation(rn[0:120, :], ps[0:120, :], AF.Silu,
                                     bias=B(f"hB{l}", 120), scale=1.0)
                st.r = rn
                if l % 3 == 2:
                    bn = st.hpool.tile([128, 2048], F16, name=f"h{st.s}_b{l}",
                                       tag=st.htag)
                    nc.vector.tensor_tensor(out=bn[0:120, :], in0=st.base[0:120, :],
                                            in1=rn[0:120, :], op=ALU.add)
                    st.base = bn
                    st.r = bn

            def emit_epilogue(st):
                g = st.s // 4
                pso = st.pspool.tile([128, 2048], F32, name=f"ps{st.s}_out",
                                     tag=st.pstag)
                for k in range(4):
                    cs = slice(k * 512, (k + 1) * 512)
                    nc.tensor.matmul(pso[0:3, cs], W(f"Wog{g}"), st.r[0:120, cs],
                                     start=True, stop=True)
                dd = epi.tile([3, 2048], F16, name=f"d{st.s}", tag="epi_d")
                nc.vector.tensor_tensor(out=dd[:], in0=pso[0:3, :], in1=st.Dsb[:],
                                        op=ALU.subtract)
                dsq = epi.tile([3, 2048], F16, name=f"dsq{st.s}", tag="epi_q")
                nc.vector.tensor_tensor(out=dsq[:], in0=dd[:], in1=dd[:],
                                        op=ALU.mult)
                nc.vector.tensor_reduce(out=accs[0:3, st.s:st.s + 1], in_=dsq[:],
                                        axis=AX.X, op=ALU.add)

            def lane_steps(supertiles, pspool, pstag, hpool, htag):
                steps = []
                for s in supertiles:
                    st = St()
                    st.s = s
                    st.pspool, st.pstag = pspool, pstag
                    st.hpool, st.htag = hpool, htag
                    steps.append(lambda st=st: emit_h1(st))
                    for l in range(15):
                        steps.append(lambda st=st, l=l: emit_hidden(st, l))
                    steps.append(lambda st=st: emit_epilogue(st))
                return steps

            laneA = lane_steps([0, 2, 4, 6, 8, 10], psA, "psA", hA, "hA")
            laneB = lane_steps([1, 3, 5, 7, 9, 11], psB, "psB", hB, "hB")
            STAG = 9
            for t in range(len(laneA) + STAG):
                if t < len(laneA):
                    laneA[t]()
                if t >= STAG:
                    laneB[t - STAG]()

            nc.sync.dma_start(out=out_d[:], in_=accs[:])

    nc.compile()
    return nc


_CACHE = {}


def _get_nc(shapes, wp, bp):
    key = tuple(sorted((k, v[0], str(v[1])) for k, v in shapes.items()))
    if key not in _CACHE:
        _CACHE[key] = _build_bass(shapes, wp, bp)
    return _CACHE[key]


def kernel(y, z, mu_params, ldiag_params, loff_params, pinn_params):
    from concourse.bass_utils import run_bass_kernel_spmd

    y = _as_np(y)
    z = _as_np(z)
    shared, wp, bp, w_out, b_out = _prep_shared(mu_params, ldiag_params,
                                                loff_params, pinn_params)
    in_maps = []
    for i in range(NCORES):
        m = dict(shared)
        m.update(_prep_core(y[i * NLOC:(i + 1) * NLOC],
                            z[i * NLOC:(i + 1) * NLOC], b_out))
        in_maps.append(m)

    shapes = {k: (tuple(v.shape), v.dtype) for k, v in in_maps[0].items()}
    nc = _get_nc(shapes, wp, bp)
    res = run_bass_kernel_spmd(nc, in_maps, core_ids=list(range(NCORES)))

    total = 0.0
    for i in range(NCORES):
        o = np.asarray(res.results[i]["out"], dtype=np.float64)
        Ssq = o[0:3, 0:12].sum()
        Sxi2 = o[0:5, 12:16].sum()
        Sent = o[0:5, 16].sum()
        core = (Sent + NLOC * 0.5 * D_XI * (1.0 + LOG2PI)
                + (1.0 / P) * (-0.5 * Sxi2 - 0.5 * D_XI * LOG2PI * NXI)
                + (1.0 / P) * (-(0.5 / NOISE ** 2) * Ssq
                               + NXI * N_TICKS * (-np.log(NOISE) - 0.5 * LOG2PI)))
        total += core
    loss = total / (NCORES * NLOC)
    return np.float32(loss)
